# revision 7
# baseline (speedup 1.0000x reference)
"""CMCL loss kernel for Trainium2 (Bass/Tile), data-parallel over 8 NeuronCores.

Reference computation (M=4 models, B=8192 samples, C=1000 classes):
    logp   = log_softmax(logits, -1)
    ce     = -logp[m, b, t[b]]                      = lse[m,b] - x_t[m,b]
    ent    = -log(C) - mean_c(log_softmax(x+eps))   = lse[m,b] - meanl[m,b] - log(C)
    loss   = ce + (sum_m ent - ent)
    min_index = argmin_m loss                        (lse cancels: argmin_m of
                                                     meanl - x_t, negated here)
    oracle_logits[b] = logits[min_index[b], b]
    new_loss = sum_b (ce-ent)_winner / B + sum ent / B

Per-core device work (B_loc = 1024 = 8 chunks x 128 partitions):
  for each (m, chunk) tile [128, 1000]:
    sexp = sum_c exp(x)          (ScalarE activation Exp + accum)
    x_t  = sum_c (iota==t)*x     (VectorE scalar_tensor_tensor + accum; exact)
    ssum = sum_c x               (tensor_scalar + accum, split across engines)
  argmin via max8/max_index on d' = x_t - ssum/1000 (= -(loss-const));
  oracle rows written straight from resident SBUF tiles with an indirect
  scatter DMA whose losing rows are pushed out-of-bounds (skipped).
  Scalar terms are returned as per-partition partials, reduced on host.
"""

import sys

if "/opt/trn_rl_repo" not in sys.path:
    sys.path.insert(0, "/opt/trn_rl_repo")

import numpy as np

import concourse.bacc as bacc
import concourse.bass as bass
import concourse.tile as tile
from concourse import mybir
from concourse.bass_utils import run_bass_kernel_spmd

M, B, C = 4, 8192, 1000
NCORES = 8
BLOC = B // NCORES          # 1024 samples per core
NCHUNK = BLOC // 128        # 8 chunks of 128 partitions
LOGC = float(np.log(np.float32(C)))
OOB = 1.0e7                 # offsets >= this are skipped by the scatter

# which engine computes the plain column sum for tile counter k = m*NCHUNK + j
# (v = VectorE, a = ScalarE, g = GpSimd) -- balance the three engines.
SUM_ENGINE = ("v", "a", "v", "v", "a", "v", "v", "a")


def _build():
    nc = bacc.Bacc("TRN2", target_bir_lowering=False, debug=False,
                   num_devices=NCORES)
    f32, i32, u32 = mybir.dt.float32, mybir.dt.int32, mybir.dt.uint32

    x_d = nc.dram_tensor("x", [M, BLOC, C], f32, kind="ExternalInput")
    t_d = nc.dram_tensor("tcol", [128, NCHUNK], f32, kind="ExternalInput")
    oracle_d = nc.dram_tensor("oracle", [BLOC, C], f32, kind="ExternalOutput")
    minidx_d = nc.dram_tensor("minidx", [128, NCHUNK], i32, kind="ExternalOutput")
    parts_d = nc.dram_tensor("partials", [128, 3], f32, kind="ExternalOutput")

    NT = M * NCHUNK
    from contextlib import ExitStack
    with tile.TileContext(nc) as tc, ExitStack() as ctx:
        consts = ctx.enter_context(tc.tile_pool(name="consts", bufs=1))
        stats = ctx.enter_context(tc.tile_pool(name="stats", bufs=1))
        xpool = ctx.enter_context(tc.tile_pool(name="x", bufs=1))
        exp_s = ctx.enter_context(tc.tile_pool(name="exp_s", bufs=2))
        stt_s = ctx.enter_context(tc.tile_pool(name="stt_s", bufs=2))
        sum_s = ctx.enter_context(tc.tile_pool(name="sum_s", bufs=2))

        # constants
        iota_i = consts.tile([128, C], i32)
        nc.gpsimd.iota(iota_i[:], pattern=[[1, C]], channel_multiplier=0)
        iota_f = consts.tile([128, C], f32)
        nc.vector.tensor_copy(iota_f[:], iota_i[:])
        prow_i = consts.tile([128, NCHUNK], i32)
        nc.gpsimd.iota(prow_i[:], pattern=[[128, NCHUNK]], channel_multiplier=1)
        prow_f = consts.tile([128, NCHUNK], f32)
        nc.vector.tensor_copy(prow_f[:], prow_i[:])
        zerob = consts.tile([128, 1], f32)
        nc.vector.memset(zerob[:], 0.0)
        tcol = consts.tile([128, NCHUNK], f32)
        nc.sync.dma_start(out=tcol[:], in_=t_d[:])

        # per-(m,chunk) stats, column c = m*NCHUNK + j
        sexp = stats.tile([128, NT], f32)
        ssum = stats.tile([128, NT], f32)
        xt = stats.tile([128, NT], f32)
        lse = stats.tile([128, NT], f32)
        dn = stats.tile([128, NCHUNK, 8], f32)   # per-chunk rows of 8 slots
        w8 = stats.tile([128, NCHUNK, 8], f32)
        wi = stats.tile([128, NCHUNK, 8], u32)
        idxf = stats.tile([128, NCHUNK], f32)
        nef = stats.tile([128, M, NCHUNK], f32)
        offf = stats.tile([128, M, NCHUNK], f32)
        offi = stats.tile([128, M, NCHUNK], i32)
        mi = stats.tile([128, NCHUNK], i32)
        parts = stats.tile([128, 3], f32)

        nc.vector.memset(dn[:], -1.0e30)

        xt_tiles = {}
        for m in range(M):
            for j in range(NCHUNK):
                c = m * NCHUNK + j
                xmj = xpool.tile([128, C], f32, name=f"x{m}_{j}", tag=f"x{m}_{j}")
                xt_tiles[(m, j)] = xmj
                nc.sync.dma_start(out=xmj[:], in_=x_d[m, j * 128:(j + 1) * 128, :])
                # sum_c exp(x)
                nc.scalar.activation(
                    out=exp_s.tile([128, C], f32, name="exp_scr", tag="exp_s"),
                    in_=xmj[:],
                    func=mybir.ActivationFunctionType.Exp,
                    bias=zerob[:],
                    accum_out=sexp[:, c:c + 1],
                )
                # x_t: sum_c (iota == t) * x   (exact gather)
                nc.vector.scalar_tensor_tensor(
                    out=stt_s.tile([128, C], f32, name="stt_scr", tag="stt_s"),
                    in0=iota_f[:],
                    scalar=tcol[:, j:j + 1],
                    in1=xmj[:],
                    op0=mybir.AluOpType.is_equal,
                    op1=mybir.AluOpType.mult,
                    accum_out=xt[:, c:c + 1],
                )
                # plain sum_c x, engine-balanced
                eng = SUM_ENGINE[c % len(SUM_ENGINE)]
                if eng == "a":
                    nc.scalar.activation(
                        out=sum_s.tile([128, C], f32, name="suma_scr", tag="sum_a"),
                        in_=xmj[:],
                        func=mybir.ActivationFunctionType.Copy,
                        accum_out=ssum[:, c:c + 1],
                    )
                else:
                    e = nc.vector if eng == "v" else nc.gpsimd
                    e.tensor_scalar(
                        sum_s.tile([128, C], f32, name="sumvg_scr", tag="sum_vg"),
                        xmj[:],
                        1.0,
                        None,
                        mybir.AluOpType.mult,
                        mybir.AluOpType.add,
                        accum_out=ssum[:, c:c + 1],
                    )

        # ---- decision math on [128, 32]-sized stats ----
        ssum_r = ssum[:].rearrange("p (m j) -> p j m", m=M)
        xt_r = xt[:].rearrange("p (m j) -> p j m", m=M)
        # dn[:, j, m] = xt - ssum/1000  (= -(loss[m] - per-sample const))
        nc.vector.scalar_tensor_tensor(
            out=dn[:, :, 0:M],
            in0=ssum_r,
            scalar=-1.0 / C,
            in1=xt_r,
            op0=mybir.AluOpType.mult,
            op1=mybir.AluOpType.add,
        )
        for j in range(NCHUNK):
            nc.vector.max(w8[:, j, :], dn[:, j, :])
            nc.vector.max_index(wi[:, j, :], w8[:, j, :], dn[:, j, :])
        nc.vector.tensor_copy(idxf[:], wi[:, :, 0])
        nc.vector.tensor_copy(mi[:], wi[:, :, 0])
        nc.sync.dma_start(out=minidx_d[:], in_=mi[:])

        for m in range(M):
            nc.vector.tensor_scalar(
                nef[:, m, :], idxf[:], float(m), None, mybir.AluOpType.not_equal)
            nc.vector.scalar_tensor_tensor(
                out=offf[:, m, :],
                in0=nef[:, m, :],
                scalar=OOB,
                in1=prow_f[:],
                op0=mybir.AluOpType.mult,
                op1=mybir.AluOpType.add,
            )
            nc.vector.tensor_copy(offi[:, m, :], offf[:, m, :])

        # partials: col0 = sum lse, col1 = sum ssum, col2 = sum d'_win
        nc.scalar.activation(
            out=lse[:], in_=sexp[:], func=mybir.ActivationFunctionType.Ln,
            bias=zerob[:], accum_out=parts[:, 0:1])
        nc.vector.tensor_reduce(
            parts[:, 1:2], ssum[:], axis=mybir.AxisListType.X,
            op=mybir.AluOpType.add)
        nc.vector.tensor_reduce(
            parts[:, 2:3], w8[:, :, 0:1], axis=mybir.AxisListType.XY,
            op=mybir.AluOpType.add)
        nc.sync.dma_start(out=parts_d[:], in_=parts[:])

        # oracle rows: scatter the winning model's resident tile rows;
        # losing rows get offsets >= OOB and are skipped.
        for m in range(M):
            for j in range(NCHUNK):
                nc.gpsimd.indirect_dma_start(
                    out=oracle_d[:],
                    out_offset=bass.IndirectOffsetOnAxis(
                        ap=offi[:, m, j:j + 1], axis=0),
                    in_=xt_tiles[(m, j)][:],
                    in_offset=None,
                    bounds_check=BLOC - 1,
                    oob_is_err=False,
                )

    nc.compile()
    return nc


_NC = None
_last_in_maps = None


def _get_nc():
    global _NC
    if _NC is None:
        _NC = _build()
    return _NC


def kernel(logits: np.ndarray, targets: np.ndarray) -> tuple:
    logits = np.ascontiguousarray(logits, dtype=np.float32)
    tgt = np.asarray(targets)

    in_maps = []
    for k in range(NCORES):
        sl = slice(k * BLOC, (k + 1) * BLOC)
        tcol = np.ascontiguousarray(
            tgt[sl].reshape(NCHUNK, 128).T.astype(np.float32))
        in_maps.append({
            "x": np.ascontiguousarray(logits[:, sl, :]),
            "tcol": tcol,
        })

    global _last_in_maps
    _last_in_maps = in_maps
    res = run_bass_kernel_spmd(_get_nc(), in_maps, core_ids=list(range(NCORES)))

    oracle = np.concatenate([res.results[k]["oracle"] for k in range(NCORES)], axis=0)
    minidx = np.concatenate(
        [res.results[k]["minidx"].T.reshape(-1) for k in range(NCORES)])

    sum_lse = 0.0
    sum_ssum = 0.0
    sum_dwin = 0.0
    for k in range(NCORES):
        p = res.results[k]["partials"].astype(np.float64)
        sum_lse += p[:, 0].sum()
        sum_ssum += p[:, 1].sum()
        sum_dwin += p[:, 2].sum()

    # winner term: sum_b (ce-ent)_win = sum_b (logC - d'_win)
    winner_sum = B * LOGC - sum_dwin
    # sum of entropies: sum lse - sum meanl - M*B*logC
    ent_sum = sum_lse - sum_ssum / C - M * B * LOGC
    new_loss = np.float32((winner_sum + ent_sum) / B)

    out_int = np.int64 if tgt.dtype == np.int64 else np.int32
    return new_loss, oracle, minidx.astype(out_int)


# revision 12
# speedup vs baseline: 1.4409x; 1.4409x over previous
"""CMCL loss kernel for Trainium2 (Bass/Tile), data-parallel over 8 NeuronCores.

Reference computation (M=4 models, B=8192 samples, C=1000 classes):
    logp   = log_softmax(logits, -1)
    ce     = -logp[m, b, t[b]]                      = lse[m,b] - x_t[m,b]
    ent    = -log(C) - mean_c(log_softmax(x+eps))   = lse[m,b] - meanl[m,b] - log(C)
    loss   = ce + (sum_m ent - ent)
    min_index = argmin_m loss                        (lse cancels: argmin_m of
                                                     meanl - x_t, negated here)
    oracle_logits[b] = logits[min_index[b], b]
    new_loss = sum_b (ce-ent)_winner / B + sum ent / B

Per-core device work (B_loc = 1024 = 8 chunks x 128 partitions):
  - x_t[m,b] (the target logits) arrive via one indirect-DMA gather with
    host-computed element offsets (exact, overlaps with the bulk load).
  - per (m, chunk) tile [128, 1000]: sexp = sum_c exp(x) on ScalarE
    (activation + accum), ssum = sum_c x on VectorE/ScalarE (tensor_scalar
    + accum), engine-balanced.
  - argmin via max8/max_index on d' = x_t - ssum/1000 (= -(loss - const)).
  - oracle rows are written straight from the resident SBUF tiles with one
    indirect scatter DMA per model; losing rows get out-of-bounds offsets
    and are skipped.  Each model scatters into its own DRAM buffer (rows
    are disjoint; the host sums the pre-zeroed buffers) so the scatters
    don't serialize on a write-after-write chain.
  - scalar terms are returned as per-partition partials, reduced on host.
"""

import sys

if "/opt/trn_rl_repo" not in sys.path:
    sys.path.insert(0, "/opt/trn_rl_repo")

import numpy as np

import concourse.bacc as bacc
import concourse.bass as bass
import concourse.tile as tile
from concourse import mybir
from concourse.bass_utils import run_bass_kernel_spmd

M, B, C = 4, 8192, 1000
NCORES = 8
BLOC = B // NCORES          # 1024 samples per core
NCHUNK = BLOC // 128        # 8 chunks of 128 partitions
NT = M * NCHUNK             # 32 tiles of [128, C] per core
LOGC = float(np.log(np.float32(C)))
OOB = 1.0e7                 # offsets >= this are skipped by the scatter

# engine for the plain column sum of tile k = m*NCHUNK + j
# (v = VectorE tensor_scalar+accum, a = ScalarE activation Copy+accum)
SUM_ENGINE = ("v", "a", "v", "a", "v", "a", "v", "a")


def _build():
    nc = bacc.Bacc("TRN2", target_bir_lowering=False, debug=False,
                   num_devices=NCORES)
    f32, i32, u32 = mybir.dt.float32, mybir.dt.int32, mybir.dt.uint32

    x_d = nc.dram_tensor("x", [M, BLOC, C], f32, kind="ExternalInput")
    t_d = nc.dram_tensor("tcol", [128, NCHUNK], f32, kind="ExternalInput")
    oracle_d = [
        nc.dram_tensor(f"oracle{m}", [BLOC, C], f32, kind="ExternalOutput")
        for m in range(M)
    ]
    minidx_d = nc.dram_tensor("minidx", [128, NCHUNK], i32, kind="ExternalOutput")
    parts_d = nc.dram_tensor("partials", [128, 3], f32, kind="ExternalOutput")

    from contextlib import ExitStack
    with tile.TileContext(nc) as tc, ExitStack() as ctx:
        consts = ctx.enter_context(tc.tile_pool(name="consts", bufs=1))
        stats = ctx.enter_context(tc.tile_pool(name="stats", bufs=1))
        xpool = ctx.enter_context(tc.tile_pool(name="x", bufs=1))
        exp_s = ctx.enter_context(tc.tile_pool(name="exp_s", bufs=2))
        sum_s = ctx.enter_context(tc.tile_pool(name="sum_s", bufs=2))
        stt_s = ctx.enter_context(tc.tile_pool(name="stt_s", bufs=2))

        # constants
        prow_i = consts.tile([128, NCHUNK], i32)
        nc.gpsimd.iota(prow_i[:], pattern=[[128, NCHUNK]], channel_multiplier=1)
        prow_f = consts.tile([128, NCHUNK], f32)
        nc.vector.tensor_copy(prow_f[:], prow_i[:])
        zerob = consts.tile([128, 1], f32)
        nc.vector.memset(zerob[:], 0.0)
        tcol = consts.tile([128, NCHUNK], f32)
        nc.sync.dma_start(out=tcol[:], in_=t_d[:])
        iota_i = consts.tile([128, C], i32)
        nc.gpsimd.iota(iota_i[:], pattern=[[1, C]], channel_multiplier=0)
        iota_f = consts.tile([128, C], f32)
        nc.vector.tensor_copy(iota_f[:], iota_i[:])

        # per-(m,chunk) stats, column c = m*NCHUNK + j
        sexp = stats.tile([128, NT], f32)
        ssum = stats.tile([128, NT], f32)
        xt = stats.tile([128, NT], f32)
        lse = stats.tile([128, NT], f32)
        dn = stats.tile([128, NCHUNK, 8], f32)   # per-chunk rows of 8 slots
        w8 = stats.tile([128, NCHUNK, 8], f32)
        wi = stats.tile([128, NCHUNK, 8], u32)
        idxf = stats.tile([128, NCHUNK], f32)
        nef = stats.tile([128, M, NCHUNK], f32)
        offf = stats.tile([128, M, NCHUNK], f32)
        offi = stats.tile([128, M, NCHUNK], i32)
        mi = stats.tile([128, NCHUNK], i32)
        parts = stats.tile([128, 3], f32)

        nc.vector.memset(dn[:], -1.0e30)

        xbig = []
        for m in range(M):
            xb = xpool.tile([128, NCHUNK, C], f32, name=f"xb{m}", tag=f"xb{m}")
            xbig.append(xb)
            for j in range(NCHUNK):
                c = m * NCHUNK + j
                nc.sync.dma_start(
                    out=xb[:, j, :], in_=x_d[m, j * 128:(j + 1) * 128, :])
                nc.scalar.activation(
                    out=exp_s.tile([128, C], f32, name="exp_scr", tag="exp_s"),
                    in_=xb[:, j, :],
                    func=mybir.ActivationFunctionType.Exp,
                    bias=zerob[:],
                    accum_out=sexp[:, c:c + 1],
                )
                nc.vector.scalar_tensor_tensor(
                    out=stt_s.tile([128, C], f32, name="stt_scr", tag="stt_s"),
                    in0=iota_f[:],
                    scalar=tcol[:, j:j + 1],
                    in1=xb[:, j, :],
                    op0=mybir.AluOpType.is_equal,
                    op1=mybir.AluOpType.mult,
                    accum_out=xt[:, c:c + 1],
                )
                if SUM_ENGINE[c % len(SUM_ENGINE)] == "a":
                    nc.scalar.activation(
                        out=sum_s.tile([128, C], f32, name="suma_scr", tag="sum_a"),
                        in_=xb[:, j, :],
                        func=mybir.ActivationFunctionType.Copy,
                        accum_out=ssum[:, c:c + 1],
                    )
                else:
                    nc.vector.tensor_scalar(
                        sum_s.tile([128, C], f32, name="sumv_scr", tag="sum_v"),
                        xb[:, j, :],
                        1.0,
                        None,
                        mybir.AluOpType.mult,
                        mybir.AluOpType.add,
                        accum_out=ssum[:, c:c + 1],
                    )

        # ---- decision math on [128, 32]-sized stats ----
        ssum_r = ssum[:].rearrange("p (m j) -> p j m", m=M)
        xt_r = xt[:].rearrange("p (m j) -> p j m", m=M)
        # dn[:, j, m] = xt - ssum/1000  (= -(loss[m] - per-sample const))
        nc.vector.scalar_tensor_tensor(
            out=dn[:, :, 0:M],
            in0=ssum_r,
            scalar=-1.0 / C,
            in1=xt_r,
            op0=mybir.AluOpType.mult,
            op1=mybir.AluOpType.add,
        )
        for j in range(NCHUNK):
            nc.vector.max(w8[:, j, :], dn[:, j, :])
            nc.vector.max_index(wi[:, j, :], w8[:, j, :], dn[:, j, :])
        nc.vector.tensor_copy(idxf[:], wi[:, :, 0])
        nc.vector.tensor_copy(mi[:], wi[:, :, 0])
        nc.sync.dma_start(out=minidx_d[:], in_=mi[:])

        for m in range(M):
            nc.vector.tensor_scalar(
                nef[:, m, :], idxf[:], float(m), None, mybir.AluOpType.not_equal)
            nc.vector.scalar_tensor_tensor(
                out=offf[:, m, :],
                in0=nef[:, m, :],
                scalar=OOB,
                in1=prow_f[:],
                op0=mybir.AluOpType.mult,
                op1=mybir.AluOpType.add,
            )
            nc.vector.tensor_copy(offi[:, m, :], offf[:, m, :])

        # partials: col0 = sum lse, col1 = sum ssum, col2 = sum d'_win
        nc.scalar.activation(
            out=lse[:], in_=sexp[:], func=mybir.ActivationFunctionType.Ln,
            bias=zerob[:], accum_out=parts[:, 0:1])
        nc.vector.tensor_reduce(
            parts[:, 1:2], ssum[:], axis=mybir.AxisListType.X,
            op=mybir.AluOpType.add)
        nc.vector.tensor_reduce(
            parts[:, 2:3], w8[:, :, 0:1], axis=mybir.AxisListType.XY,
            op=mybir.AluOpType.add)
        nc.sync.dma_start(out=parts_d[:], in_=parts[:])

        # oracle rows: one scatter per model from its resident tiles into its
        # own DRAM buffer; losing rows get OOB offsets and are skipped.
        for j in range(NCHUNK):
            for m in range(M):
                nc.gpsimd.indirect_dma_start(
                    out=oracle_d[m][:],
                    out_offset=bass.IndirectOffsetOnAxis(
                        ap=offi[:, m, j:j + 1], axis=0),
                    in_=xbig[m][:, j, :],
                    in_offset=None,
                    bounds_check=BLOC - 1,
                    oob_is_err=False,
                )

    nc.compile()
    return nc


_NC = None
_last_in_maps = None


def _get_nc():
    global _NC
    if _NC is None:
        _NC = _build()
    return _NC


def kernel(logits: np.ndarray, targets: np.ndarray) -> tuple:
    logits = np.ascontiguousarray(logits, dtype=np.float32)
    tgt = np.asarray(targets)
    t64 = tgt.astype(np.int64)

    in_maps = []
    for k in range(NCORES):
        sl = slice(k * BLOC, (k + 1) * BLOC)
        tcol = np.ascontiguousarray(
            t64[sl].reshape(NCHUNK, 128).T.astype(np.float32))
        in_maps.append({
            "x": np.ascontiguousarray(logits[:, sl, :]),
            "tcol": tcol,
        })

    global _last_in_maps
    _last_in_maps = in_maps
    res = run_bass_kernel_spmd(_get_nc(), in_maps, core_ids=list(range(NCORES)))

    oracle = np.concatenate(
        [sum(res.results[k][f"oracle{m}"] for m in range(M))
         for k in range(NCORES)], axis=0)
    minidx = np.concatenate(
        [res.results[k]["minidx"].T.reshape(-1) for k in range(NCORES)])

    sum_lse = 0.0
    sum_ssum = 0.0
    sum_dwin = 0.0
    for k in range(NCORES):
        p = res.results[k]["partials"].astype(np.float64)
        sum_lse += p[:, 0].sum()
        sum_ssum += p[:, 1].sum()
        sum_dwin += p[:, 2].sum()

    # winner term: sum_b (ce-ent)_win = sum_b (logC - d'_win)
    winner_sum = B * LOGC - sum_dwin
    # sum of entropies: sum lse - sum meanl - M*B*logC
    ent_sum = sum_lse - sum_ssum / C - M * B * LOGC
    new_loss = np.float32((winner_sum + ent_sum) / B)

    out_int = np.int64 if tgt.dtype == np.int64 else np.int32
    return new_loss, oracle, minidx.astype(out_int)


# revision 14
# speedup vs baseline: 1.9462x; 1.3507x over previous
"""CMCL loss kernel for Trainium2 (Bass/Tile), data-parallel over 8 NeuronCores.

Reference computation (M=4 models, B=8192 samples, C=1000 classes):
    logp   = log_softmax(logits, -1)
    ce     = -logp[m, b, t[b]]                      = lse[m,b] - x_t[m,b]
    ent    = -log(C) - mean_c(log_softmax(x+eps))   = lse[m,b] - meanl[m,b] - log(C)
    loss   = ce + (sum_m ent - ent)
    min_index = argmin_m loss                        (lse cancels: argmin_m of
                                                     meanl - x_t, negated here)
    oracle_logits[b] = logits[min_index[b], b]
    new_loss = sum_b (ce-ent)_winner / B + sum ent / B

Per-core device work (B_loc = 1024 = 8 chunks x 128 partitions), chunk-major
so each chunk's decision + oracle scatter overlaps later chunks' compute:
  - per (m, chunk) tile [128, 1000]: sexp = sum_c exp(x) on ScalarE
    (activation + accum), x_t = sum_c (iota==t)*x on VectorE
    (scalar_tensor_tensor + accum; exact), ssum = sum_c x on
    VectorE/ScalarE (engine-balanced).
  - per chunk: argmin over m via max8/max_index on d' = x_t - ssum/1000
    (= -(loss - per-sample const)); then one indirect scatter per model
    writes the winning rows straight from the resident SBUF tiles into
    that model's own DRAM buffer (losing rows get out-of-bounds offsets
    and are skipped; per-model buffers keep the scatters off one
    write-after-write chain).  The host sums the pre-zeroed buffers.
  - scalar terms are returned as per-partition partials, reduced on host.
"""

import sys

if "/opt/trn_rl_repo" not in sys.path:
    sys.path.insert(0, "/opt/trn_rl_repo")

import numpy as np

import concourse.bacc as bacc
import concourse.bass as bass
import concourse.tile as tile
from concourse import mybir
from concourse.bass_utils import run_bass_kernel_spmd

M, B, C = 4, 8192, 1000
NCORES = 8
BLOC = B // NCORES          # 1024 samples per core
NCHUNK = BLOC // 128        # 8 chunks of 128 partitions
NT = M * NCHUNK             # 32 tiles of [128, C] per core
LOGC = float(np.log(np.float32(C)))
OOB = 1.0e7                 # offsets >= this are skipped by the scatter

# engine for the plain column sum of tile k = m*NCHUNK + j
# (v = VectorE tensor_scalar+accum, a = ScalarE activation Copy+accum)
SUM_ENGINE = ("v", "a", "a", "v", "a", "v", "a", "a")


def _build():
    nc = bacc.Bacc("TRN2", target_bir_lowering=False, debug=False,
                   num_devices=NCORES)
    f32, i32, u32 = mybir.dt.float32, mybir.dt.int32, mybir.dt.uint32

    x_d = nc.dram_tensor("x", [M, BLOC, C], f32, kind="ExternalInput")
    t_d = nc.dram_tensor("tcol", [128, NCHUNK], f32, kind="ExternalInput")
    oracle_d = [
        nc.dram_tensor(f"oracle{m}", [BLOC, C], f32, kind="ExternalOutput")
        for m in range(M)
    ]
    minidx_d = nc.dram_tensor("minidx", [128, NCHUNK], i32, kind="ExternalOutput")
    parts_d = nc.dram_tensor("partials", [128, 3], f32, kind="ExternalOutput")

    from contextlib import ExitStack
    with tile.TileContext(nc) as tc, ExitStack() as ctx:
        consts = ctx.enter_context(tc.tile_pool(name="consts", bufs=1))
        stats = ctx.enter_context(tc.tile_pool(name="stats", bufs=1))
        xpool = ctx.enter_context(tc.tile_pool(name="x", bufs=1))
        exp_s = ctx.enter_context(tc.tile_pool(name="exp_s", bufs=2))
        sum_s = ctx.enter_context(tc.tile_pool(name="sum_s", bufs=2))
        stt_s = ctx.enter_context(tc.tile_pool(name="stt_s", bufs=2))

        # constants
        prow_i = consts.tile([128, NCHUNK], i32)
        nc.gpsimd.iota(prow_i[:], pattern=[[128, NCHUNK]], channel_multiplier=1)
        prow_f = consts.tile([128, NCHUNK], f32)
        nc.vector.tensor_copy(prow_f[:], prow_i[:])
        zerob = consts.tile([128, 1], f32)
        nc.vector.memset(zerob[:], 0.0)
        tcol = consts.tile([128, NCHUNK], f32)
        nc.sync.dma_start(out=tcol[:], in_=t_d[:])
        iota_i = consts.tile([128, C], i32)
        nc.gpsimd.iota(iota_i[:], pattern=[[1, C]], channel_multiplier=0)
        iota_f = consts.tile([128, C], f32)
        nc.vector.tensor_copy(iota_f[:], iota_i[:])
        mconst_i = consts.tile([128, M], i32)
        nc.gpsimd.iota(mconst_i[:], pattern=[[1, M]], channel_multiplier=0)
        mconst = consts.tile([128, M], f32)
        nc.vector.tensor_copy(mconst[:], mconst_i[:])

        # per-(m,chunk) stats, column c = m*NCHUNK + j
        sexp = stats.tile([128, NT], f32)
        ssum = stats.tile([128, NT], f32)
        xt = stats.tile([128, NT], f32)
        lse = stats.tile([128, NT], f32)
        dn = stats.tile([128, NCHUNK, 8], f32)   # per-chunk rows of 8 slots
        w8 = stats.tile([128, NCHUNK, 8], f32)
        wi = stats.tile([128, NCHUNK, 8], u32)
        idxf = stats.tile([128, NCHUNK], f32)
        nef = stats.tile([128, M, NCHUNK], f32)
        offf = stats.tile([128, M, NCHUNK], f32)
        offi = stats.tile([128, M, NCHUNK], i32)
        mi = stats.tile([128, NCHUNK], i32)
        parts = stats.tile([128, 3], f32)

        nc.vector.memset(dn[:], -1.0e30)

        xbig = [
            xpool.tile([128, NCHUNK, C], f32, name=f"xb{m}", tag=f"xb{m}")
            for m in range(M)
        ]

        ssum_r = ssum[:].rearrange("p (m j) -> p j m", m=M)
        xt_r = xt[:].rearrange("p (m j) -> p j m", m=M)

        for j in range(NCHUNK):
            for m in range(M):
                c = m * NCHUNK + j
                xb = xbig[m]
                nc.sync.dma_start(
                    out=xb[:, j, :], in_=x_d[m, j * 128:(j + 1) * 128, :])
                nc.scalar.activation(
                    out=exp_s.tile([128, C], f32, name="exp_scr", tag="exp_s"),
                    in_=xb[:, j, :],
                    func=mybir.ActivationFunctionType.Exp,
                    bias=zerob[:],
                    accum_out=sexp[:, c:c + 1],
                )
                nc.vector.scalar_tensor_tensor(
                    out=stt_s.tile([128, C], f32, name="stt_scr", tag="stt_s"),
                    in0=iota_f[:],
                    scalar=tcol[:, j:j + 1],
                    in1=xb[:, j, :],
                    op0=mybir.AluOpType.is_equal,
                    op1=mybir.AluOpType.mult,
                    accum_out=xt[:, c:c + 1],
                )
                if SUM_ENGINE[c % len(SUM_ENGINE)] == "a":
                    nc.scalar.activation(
                        out=sum_s.tile([128, C], f32, name="suma_scr", tag="sum_a"),
                        in_=xb[:, j, :],
                        func=mybir.ActivationFunctionType.Copy,
                        accum_out=ssum[:, c:c + 1],
                    )
                else:
                    nc.vector.tensor_scalar(
                        sum_s.tile([128, C], f32, name="sumv_scr", tag="sum_v"),
                        xb[:, j, :],
                        1.0,
                        None,
                        mybir.AluOpType.mult,
                        mybir.AluOpType.add,
                        accum_out=ssum[:, c:c + 1],
                    )

            # ---- decision + scatter for chunk j (overlaps later chunks) ----
            # dn[:, j, m] = xt - ssum/1000  (= -(loss[m] - per-sample const))
            nc.vector.scalar_tensor_tensor(
                out=dn[:, j, 0:M],
                in0=ssum_r[:, j, :],
                scalar=-1.0 / C,
                in1=xt_r[:, j, :],
                op0=mybir.AluOpType.mult,
                op1=mybir.AluOpType.add,
            )
            nc.vector.max(w8[:, j, :], dn[:, j, :])
            nc.vector.max_index(wi[:, j, :], w8[:, j, :], dn[:, j, :])
            nc.vector.tensor_copy(idxf[:, j:j + 1], wi[:, j, 0:1])
            nc.vector.tensor_copy(mi[:, j:j + 1], wi[:, j, 0:1])
            nc.vector.tensor_tensor(
                out=nef[:, :, j],
                in0=idxf[:, j:j + 1].to_broadcast([128, M]),
                in1=mconst[:],
                op=mybir.AluOpType.not_equal,
            )
            nc.vector.scalar_tensor_tensor(
                out=offf[:, :, j],
                in0=nef[:, :, j],
                scalar=OOB,
                in1=prow_f[:, j:j + 1].to_broadcast([128, M]),
                op0=mybir.AluOpType.mult,
                op1=mybir.AluOpType.add,
            )
            nc.vector.tensor_copy(offi[:, :, j], offf[:, :, j])
            for m in range(M):
                nc.gpsimd.indirect_dma_start(
                    out=oracle_d[m][:],
                    out_offset=bass.IndirectOffsetOnAxis(
                        ap=offi[:, m, j:j + 1], axis=0),
                    in_=xbig[m][:, j, :],
                    in_offset=None,
                    bounds_check=BLOC - 1,
                    oob_is_err=False,
                )

        nc.sync.dma_start(out=minidx_d[:], in_=mi[:])

        # partials: col0 = sum lse, col1 = sum ssum, col2 = sum d'_win
        nc.scalar.activation(
            out=lse[:], in_=sexp[:], func=mybir.ActivationFunctionType.Ln,
            bias=zerob[:], accum_out=parts[:, 0:1])
        nc.vector.tensor_reduce(
            parts[:, 1:2], ssum[:], axis=mybir.AxisListType.X,
            op=mybir.AluOpType.add)
        nc.vector.tensor_reduce(
            parts[:, 2:3], w8[:, :, 0:1], axis=mybir.AxisListType.XY,
            op=mybir.AluOpType.add)
        nc.sync.dma_start(out=parts_d[:], in_=parts[:])

    nc.compile()
    return nc


_NC = None
_last_in_maps = None


def _get_nc():
    global _NC
    if _NC is None:
        _NC = _build()
    return _NC


def kernel(logits: np.ndarray, targets: np.ndarray) -> tuple:
    logits = np.ascontiguousarray(logits, dtype=np.float32)
    tgt = np.asarray(targets)
    t64 = tgt.astype(np.int64)

    in_maps = []
    for k in range(NCORES):
        sl = slice(k * BLOC, (k + 1) * BLOC)
        tcol = np.ascontiguousarray(
            t64[sl].reshape(NCHUNK, 128).T.astype(np.float32))
        in_maps.append({
            "x": np.ascontiguousarray(logits[:, sl, :]),
            "tcol": tcol,
        })

    global _last_in_maps
    _last_in_maps = in_maps
    res = run_bass_kernel_spmd(_get_nc(), in_maps, core_ids=list(range(NCORES)))

    oracle = np.concatenate(
        [sum(res.results[k][f"oracle{m}"] for m in range(M))
         for k in range(NCORES)], axis=0)
    minidx = np.concatenate(
        [res.results[k]["minidx"].T.reshape(-1) for k in range(NCORES)])

    sum_lse = 0.0
    sum_ssum = 0.0
    sum_dwin = 0.0
    for k in range(NCORES):
        p = res.results[k]["partials"].astype(np.float64)
        sum_lse += p[:, 0].sum()
        sum_ssum += p[:, 1].sum()
        sum_dwin += p[:, 2].sum()

    # winner term: sum_b (ce-ent)_win = sum_b (logC - d'_win)
    winner_sum = B * LOGC - sum_dwin
    # sum of entropies: sum lse - sum meanl - M*B*logC
    ent_sum = sum_lse - sum_ssum / C - M * B * LOGC
    new_loss = np.float32((winner_sum + ent_sum) / B)

    out_int = np.int64 if tgt.dtype == np.int64 else np.int32
    return new_loss, oracle, minidx.astype(out_int)
